# revision 58
# baseline (speedup 1.0000x reference)
"""Trainium2 Bass kernel for GQA multi-head attention with RoPE.

Sharding: tensor-parallel over heads. Core c owns q-heads 4c..4c+3 and
kv-head c. Each core computes its QKV projection slice, RoPE, causal
attention for its heads, and a partial output projection
(attn_out_local @ Wo[:, local].T). The host sums the 8 partial y's.

Device layouts (per core):
  xT      [C, B*T]   x transposed (replicated)
  wcatT   [C, 384]   [Wq_loc | Wk_loc | Wv_loc].T
  qT/kT   [d, tok]   head-dim on partitions ("T-layout")
  scores  S^T [tk partitions, tq free] so no transposes are needed:
          exp(S^T) feeds P@V directly as the moving operand with
          v_aug = [v | ones] stationary; the ones row yields softmax
          row-sums in partition 64 of the PV accumulator.
  y       [B*T, C]   partial; host adds the 8 partials + bo.

Matmul operands are float32r (fp32 with 11-bit mantissa, full PE rate
at N>=256); the QKV projection inputs (x, Wqkv) are fp16 (10-bit
mantissa, halves the HBM-bound x traffic). PSUM accumulation is fp32.
"""

import sys

sys.path.insert(0, "/opt/trn_rl_repo")

import numpy as np

import bass_rust
import concourse.bass as bass
import concourse.tile as tile
from concourse import mybir
from concourse.bass_utils import run_bass_kernel_spmd

B, T, C = 2, 2048, 2048
H, KVH, D = 32, 8, 64
NTOK = B * T                 # 4096
HPC = H // 8                 # 4 q heads per core
QL = HPC * D                 # 256 local q dims
KC = C // 128                # 16 contraction chunks
SCALE = float(D) ** -0.5

F32 = mybir.dt.float32
F32R = mybir.dt.float32r
F16 = mybir.dt.float16
AF = mybir.ActivationFunctionType

_NC_CACHE = {}


def _split_waits(nc, limit=1):
    """Walrus in this toolchain allows only one sync-wait per instruction.

    Tile emits instructions with several sem waits (drain/barrier, phase
    boundaries). Hoist the excess onto same-engine NoOps inserted right
    before the instruction — program order on the engine queue preserves
    the wait semantics.
    """
    ctr = 0
    for f in nc.m.functions:
        for blk in f.blocks:
            out = []
            changed = False
            for inst in list(blk.instructions):
                si = inst.sync_info
                if si is not None and len(si.on_wait) > limit:
                    waits = list(si.on_wait)
                    keep, excess = waits[:limit], waits[limit:]
                    for i in range(0, len(excess), limit):
                        ctr += 1
                        nop = mybir.InstNoOp(
                            name=f"I-wsplit-{ctr}", ins=[], outs=[]
                        )
                        nop.engine = inst.engine
                        nop.sync_info = bass_rust.SyncInfo(
                            on_wait=excess[i : i + limit], on_update=[]
                        )
                        out.append(nop)
                        changed = True
                    inst.sync_info = bass_rust.SyncInfo(
                        on_wait=keep, on_update=list(si.on_update)
                    )
                out.append(inst)
            if changed:
                blk.instructions = out
    return ctr


def build_nc():
    nc = bass.Bass(trn_type="TRN2")

    xT_d = nc.dram_tensor("xT", [C, NTOK], F16, kind="ExternalInput")
    wcat_d = nc.dram_tensor("wcatT", [C, 384], F16, kind="ExternalInput")
    bias_d = nc.dram_tensor("bqkv", [128, 3], F32, kind="ExternalInput")
    wo_d = nc.dram_tensor("woR", [QL, C], F16, kind="ExternalInput")
    cos_d = nc.dram_tensor("cos2", [128, NTOK], F32R, kind="ExternalInput")
    sin_d = nc.dram_tensor("sin2", [128, NTOK], F32R, kind="ExternalInput")
    pmat_d = nc.dram_tensor("pmat", [128, 128], F32R, kind="ExternalInput")
    dneg_d = nc.dram_tensor("dneg", [128, 128], F32R, kind="ExternalInput")
    id128_d = nc.dram_tensor("id128", [128, 128], F32R, kind="ExternalInput")
    dmask_d = nc.dram_tensor("dmask", [128, 128], F16, kind="ExternalInput")
    ident_d = nc.dram_tensor("ident2", [128, 64], F32R, kind="ExternalInput")
    vones_d = nc.dram_tensor(
        "vones", [128, 2 * (T // 128)], F16, kind="ExternalInput"
    )
    y_d = nc.dram_tensor("y", [NTOK, C], F16, kind="ExternalOutput")

    with tile.TileContext(nc) as tc:
        with (
            tc.tile_pool(name="consts", bufs=1) as consts,
            tc.tile_pool(name="xs", bufs=8) as xs_pool,
            tc.tile_pool(name="acts", bufs=1) as acts,
            tc.tile_pool(name="big", bufs=2) as big,
            tc.tile_pool(name="tmp", bufs=2) as tmp_pool,
            tc.tile_pool(name="es", bufs=3) as es_pool,
            tc.tile_pool(name="rows", bufs=2) as rows,
            tc.tile_pool(name="ibc", bufs=3) as ibc_pool,
            tc.tile_pool(name="ocp", bufs=10) as ocp_pool,
            tc.tile_pool(name="onorm", bufs=1) as on_pool,
            tc.tile_pool(name="ystage", bufs=2) as y_pool,
            tc.tile_pool(name="psA", bufs=2, space="PSUM") as psA,
            tc.tile_pool(name="psB", bufs=1, space="PSUM") as psB,
            tc.tile_pool(name="fil", bufs=1, space="PSUM") as fil,
        ):
            # ---- constants needed by phase 1's first group ----
            # wo / mbias / cos / sin are deliberately NOT loaded here: the
            # first QKV matmul stalls on everything queued ahead of its xt
            # tile, so only group-0 prerequisites go first. wcat is one tile
            # per k chunk so matmul k waits on exactly its own DMA.
            wcat_ks = [
                consts.tile([128, 384], F16, tag=f"wc{k}", name=f"wcat{k}")
                for k in range(KC)
            ]
            bias_sb = consts.tile([128, 3], F32, tag="bias")
            pmat_sb = consts.tile([128, 128], F32R, tag="pmat")
            ident_sb = consts.tile([128, 64], F32R, tag="ident")

            wo_sb = consts.tile([128, 2, C], F16, tag="wo")
            dneg_sb = consts.tile([128, 128], F32R, tag="dneg")
            id128_sb = consts.tile([128, 128], F32R, tag="id128")
            dmask_sb = consts.tile([128, 128], F16, tag="dmask")

            qT01 = acts.tile([128, NTOK], F32R, tag="qT01")
            qT23 = acts.tile([128, NTOK], F32R, tag="qT23")
            kvT = acts.tile([128, NTOK], F32R, tag="kvT")
            ktdup = acts.tile([128, NTOK], F32R, tag="ktdup")
            vaug = acts.tile([128, 2 * (T // 128), 65], F16, tag="vaug")

            def cs_pair(tok0, width):
                cg = big.tile([128, 1024], F32R, tag="cs", bufs=4,
                              name="cosg")
                sg = big.tile([128, 1024], F32R, tag="cs", bufs=4,
                              name="sing")
                nc.sync.dma_start(
                    out=cg[:, :width], in_=cos_d[:, tok0 : tok0 + width]
                )
                nc.sync.dma_start(
                    out=sg[:, :width], in_=sin_d[:, tok0 : tok0 + width]
                )
                return cg[:, :width], sg[:, :width]

            # ---- phase 0: PE warmup ----
            # ~40 back-to-back tiny fp16 matmuls on a memset tile keep the
            # PE HAM activity window busy from t=0, so the first real QKV
            # matmuls run at 2.4 GHz instead of the cold 1.2 GHz ramp.
            wtile = consts.tile([128, 128], F16, tag="warm")
            nc.vector.memset(wtile, 0.0)
            wps = psA.tile([128, 128], F32, tag="a")
            for _ in range(40):
                nc.tensor.matmul(
                    wps, wtile, wtile, start=True, stop=True
                )

            # ---- phase 1: QKV projection + RoPE + v-transpose, fused ----
            # Processing 1024-token groups keeps the PE stream dense: the
            # RoPE rotate-matmuls and v-transposes of group g interleave
            # with the QKV matmuls of group g+1, so the HAM never
            # re-throttles between phases.
            qkv_dst = [qT01, qT23, kvT]
            NJ = T // 128  # 16
            for ng in range(2):  # b0 groups only; b1 QKV is interleaved
                base = 1024 * ng  # under attention-b0 (see gen_b1_qkv)
                ps0 = psA.tile([128, 1024], F32, tag="a")
                ps1 = psA.tile([128, 1024], F32, tag="a")
                ps2 = psB.tile([128, 1024], F32, tag="o")
                pss = [ps0, ps1, ps2]
                for k2 in range(KC // 2):
                    if ng == 0:
                        for k in (2 * k2, 2 * k2 + 1):
                            nc.sync.dma_start(
                                out=wcat_ks[k],
                                in_=wcat_d[128 * k : 128 * (k + 1), :],
                            )
                    xt2 = xs_pool.tile([128, 2, 1024], F16, tag="xs", bufs=4)
                    for kk in range(2):
                        k = 2 * k2 + kk
                        nc.sync.dma_start(
                            out=xt2[:, kk, :],
                            in_=xT_d[128 * k : 128 * (k + 1),
                                     base : base + 1024],
                        )
                    for kk in range(2):
                        k = 2 * k2 + kk
                        for m in range(3):
                            for c2 in range(2):
                                nc.tensor.matmul(
                                    pss[m][:, 512 * c2 : 512 * (c2 + 1)],
                                    wcat_ks[k][:, 128 * m : 128 * (m + 1)],
                                    xt2[:, kk, 512 * c2 : 512 * (c2 + 1)],
                                    start=(k == 0),
                                    stop=(k == KC - 1),
                                )
                cos_g, sin_g = cs_pair(base, 1024)
                if ng == 0:
                    nc.sync.dma_start(out=bias_sb, in_=bias_d[:, :])
                    nc.sync.dma_start(out=pmat_sb, in_=pmat_d[:, :])
                    nc.sync.dma_start(out=ident_sb, in_=ident_d[:, :])
                    nc.sync.dma_start(
                        out=vaug[:, :, 64:65], in_=vones_d[:, :].unsqueeze(2)
                    )
                for m in range(3):
                    nc.scalar.activation(
                        out=qkv_dst[m][:, base : base + 1024],
                        in_=pss[m],
                        func=AF.Identity,
                        bias=bias_sb[:, m : m + 1],
                        scale=1.0,
                    )
                # RoPE for this token group (token-pointwise)
                for dst, rn in ((qT01, 128), (qT23, 128), (kvT, 64)):
                    rot = psA.tile([128, 1024], F32, tag="a")
                    for c2 in range(2):
                        nc.tensor.matmul(
                            rot[:rn, 512 * c2 : 512 * (c2 + 1)],
                            pmat_sb[:rn, :rn],
                            dst[:rn, base + 512 * c2 : base + 512 * (c2 + 1)],
                            start=True,
                            stop=True,
                        )
                    tmp = tmp_pool.tile([128, 1024], F32, tag="tmp")
                    nc.vector.tensor_mul(
                        tmp[:rn], rot[:rn, :], sin_g[:rn, :]
                    )
                    nc.vector.tensor_mul(
                        dst[:rn, base : base + 1024],
                        dst[:rn, base : base + 1024],
                        cos_g[:rn, :],
                    )
                    nc.vector.tensor_add(
                        dst[:rn, base : base + 1024],
                        dst[:rn, base : base + 1024],
                        tmp[:rn],
                    )
                # duplicate this group's roped kT into partitions 64:128
                # (overlaps with the next group instead of one serial copy)
                nc.sync.dma_start(
                    out=ktdup[64:128, base : base + 1024],
                    in_=kvT[0:64, base : base + 1024],
                )
                # v transposes for this token group (v is not roped)
                b2 = ng // 2
                for jj in range(8):
                    jt = (ng % 2) * 8 + jj
                    vps = psB.tile([128, 64], F32R, tag="o")
                    nc.tensor.transpose(
                        vps,
                        kvT[64:128, T * b2 + 128 * jt : T * b2 + 128 * (jt + 1)],
                        ident_sb[64:128, :],
                    )
                    # ACT is idle during phase 1; keep DVE free for RoPE
                    nc.scalar.copy(vaug[:, b2 * NJ + jt, 0:64], vps)

            # attention/phase-4 constants: queued behind phase 1's loads so
            # they never delay the first QKV matmuls
            nc.sync.dma_start(out=dneg_sb, in_=dneg_d[:, :])
            nc.sync.dma_start(out=id128_sb, in_=id128_d[:, :])
            nc.sync.dma_start(out=dmask_sb, in_=dmask_d[:, :])
            for k in range(2):
                nc.sync.dma_start(
                    out=wo_sb[:, k, :], in_=wo_d[128 * k : 128 * (k + 1), :]
                )

            # ---- phase 3: attention, per (batch, head, tq-half) ----
            OT0 = big.tile([128, NTOK], F16, tag="ot0", bufs=1)
            OT1 = big.tile([128, NTOK], F16, tag="ot1", bufs=1)
            OT = [OT0, OT1]

            def gen_b1_qkv():
                # b1's QKV+RoPE+v-transpose as a stream of small emission
                # units, pumped one per attention-b0 j-step so the PE FIFO
                # alternates [attention j][qkv chunk] and stays dense while
                # ACT grinds the exp pipeline. PSUM: dedicated 2-bank tag
                # ("f"), two-pass per 512-token halfgroup (q01/q23 pass,
                # then kv pass re-reading the same xt tiles).
                for hb in range(4):
                    tok0 = T + 512 * hb
                    cos_h, sin_h = cs_pair(tok0, 512)
                    fq = fil.tile([128, 2, 512], F32, tag="f", name=f"fq{hb}")
                    xts = []
                    for cu in range(8):
                        xt2 = xs_pool.tile(
                            [128, 2, 512], F16, tag="xsb", bufs=8,
                            name=f"xb{hb}_{cu}",
                        )
                        for kk in range(2):
                            k = 2 * cu + kk
                            nc.sync.dma_start(
                                out=xt2[:, kk, :],
                                in_=xT_d[128 * k : 128 * (k + 1),
                                         tok0 : tok0 + 512],
                            )
                        xts.append(xt2)
                        for kk in range(2):
                            k = 2 * cu + kk
                            for m in range(2):
                                nc.tensor.matmul(
                                    fq[:, m, :],
                                    wcat_ks[k][:, 128 * m : 128 * (m + 1)],
                                    xt2[:, kk, :],
                                    start=(k == 0), stop=(k == KC - 1),
                                )
                        yield
                    for m in range(2):
                        nc.vector.tensor_scalar_add(
                            [qT01, qT23][m][:, tok0 : tok0 + 512],
                            fq[:, m, :],
                            bias_sb[:, m : m + 1],
                        )
                    yield
                    fkv = fil.tile(
                        [128, 2, 512], F32, tag="f", name=f"fkv{hb}"
                    )
                    for cu in range(8):
                        for kk in range(2):
                            k = 2 * cu + kk
                            nc.tensor.matmul(
                                fkv[:, 0, :],
                                wcat_ks[k][:, 256:384],
                                xts[cu][:, kk, :],
                                start=(k == 0), stop=(k == KC - 1),
                            )
                        if cu % 2 == 1:
                            yield
                    nc.vector.tensor_scalar_add(
                        kvT[:, tok0 : tok0 + 512],
                        fkv[:, 0, :],
                        bias_sb[:, 2:3],
                    )
                    yield
                    # v transposes (v is not roped; kvT[64:128] = v)
                    for vi in range(2):
                        vt = fil.tile(
                            [128, 2, 64], F32R, tag="f", name=f"vt{hb}{vi}"
                        )
                        for w2 in range(2):
                            jt = 4 * hb + 2 * vi + w2
                            nc.tensor.transpose(
                                vt[:, w2, :],
                                kvT[64:128, T + 128 * jt : T + 128 * (jt + 1)],
                                ident_sb[64:128, :],
                            )
                            nc.vector.tensor_copy(
                                vaug[:, NJ + jt, 0:64], vt[:, w2, :]
                            )
                        yield
                    # RoPE for q01/q23
                    rotq = fil.tile(
                        [128, 2, 512], F32, tag="f", name=f"rq{hb}"
                    )
                    for m in range(2):
                        nc.tensor.matmul(
                            rotq[:, m, :],
                            pmat_sb,
                            [qT01, qT23][m][:, tok0 : tok0 + 512],
                            start=True, stop=True,
                        )
                    yield
                    for m in range(2):
                        dst = [qT01, qT23][m]
                        tmp = tmp_pool.tile([128, 512], F32, tag="tmp")
                        nc.vector.tensor_mul(tmp, rotq[:, m, :], sin_h)
                        nc.vector.tensor_mul(
                            dst[:, tok0 : tok0 + 512],
                            dst[:, tok0 : tok0 + 512], cos_h,
                        )
                        nc.vector.tensor_add(
                            dst[:, tok0 : tok0 + 512],
                            dst[:, tok0 : tok0 + 512], tmp,
                        )
                        yield
                    # RoPE for k (rows 0:64 of kvT)
                    rotk = fil.tile([128, 512], F32, tag="f", name=f"rk{hb}")
                    nc.tensor.matmul(
                        rotk[:64, :], pmat_sb[:64, :64],
                        kvT[0:64, tok0 : tok0 + 512],
                        start=True, stop=True,
                    )
                    yield
                    tmp = tmp_pool.tile([128, 512], F32, tag="tmp")
                    nc.vector.tensor_mul(
                        tmp[:64], rotk[:64, :], sin_h[:64, :]
                    )
                    nc.vector.tensor_mul(
                        kvT[0:64, tok0 : tok0 + 512],
                        kvT[0:64, tok0 : tok0 + 512], cos_h[:64, :],
                    )
                    nc.vector.tensor_add(
                        kvT[0:64, tok0 : tok0 + 512],
                        kvT[0:64, tok0 : tok0 + 512], tmp[:64],
                    )
                    yield
                    nc.sync.dma_start(
                        out=ktdup[64:128, tok0 : tok0 + 512],
                        in_=kvT[0:64, tok0 : tok0 + 512],
                    )
                    yield

            b1_units = gen_b1_qkv()

            def pump(n=1):
                for _ in range(n):
                    if next(b1_units, "done") == "done":
                        break

            half_ys = {}

            def out_proj_half(mt, half, act_free):
                # Half of one token block's output projection: two 512-col
                # chunks accumulated in a 2-bank "f" tile (free outside the
                # b1-QKV stretch), copies to a shared staging tile, and on
                # the second half one [128, 2048] DMA. Interleaved halves
                # are emitted at separate attention j-steps so the f-ring's
                # draw-waits-copy dependency hides behind an attention step
                # instead of stalling the PE FIFO.
                if half == 0:
                    ys = y_pool.tile([128, C], F16, tag="ys")
                    half_ys[mt] = ys
                else:
                    ys = half_ys.pop(mt)
                fp = fil.tile(
                    [128, 2, 512], F32, tag="f", name=f"yp{mt}_{half}"
                )
                for c2 in range(2):
                    ncc = 2 * half + c2
                    for kq in range(2):
                        nc.tensor.matmul(
                            fp[:, c2, :],
                            OT[kq][:, 128 * mt : 128 * (mt + 1)],
                            wo_sb[:, kq, 512 * ncc : 512 * (ncc + 1)],
                            start=(kq == 0),
                            stop=(kq == 1),
                        )
                for c2 in range(2):
                    ncc = 2 * half + c2
                    ysl = ys[:, 512 * ncc : 512 * (ncc + 1)]
                    if act_free and c2 == 0:
                        nc.scalar.copy(ysl, fp[:, c2, :])
                    else:
                        nc.vector.tensor_copy(ysl, fp[:, c2, :])
                if half == 1:
                    nc.sync.dma_start(
                        out=y_d[128 * mt : 128 * (mt + 1), :], in_=ys,
                    )

            def out_proj(mt, act_free):
                # full-unit path for the tail: psA's "a" ring has no sps
                # draws there, so the chunk accumulators double-buffer
                ys = y_pool.tile([128, C], F16, tag="ys")
                for ncc in range(C // 512):
                    yp = psA.tile([128, 512], F32, tag="a")
                    for kq in range(2):
                        nc.tensor.matmul(
                            yp,
                            OT[kq][:, 128 * mt : 128 * (mt + 1)],
                            wo_sb[:, kq, 512 * ncc : 512 * (ncc + 1)],
                            start=(kq == 0),
                            stop=(kq == 1),
                        )
                    ysl = ys[:, 512 * ncc : 512 * (ncc + 1)]
                    # GPSIMD cannot access PSUM, so the copies are split
                    # between ACT and DVE in the tail, DVE-only while ACT
                    # is saturated with attention exps.
                    if act_free and ncc % 2 == 0:
                        nc.scalar.copy(ysl, yp)
                    else:
                        nc.vector.tensor_copy(ysl, yp)
                nc.sync.dma_start(
                    out=y_d[128 * mt : 128 * (mt + 1), :], in_=ys,
                )

            # Normalize ops are queued as small closures and dripped out
            # one per attention j-step: an 8-norm burst emitted right at an
            # hp boundary waits multi-us on the rowsum DRAM roundtrip and
            # head-of-line-blocks the DVE FIFO (and, through the first
            # out-proj matmul, the PE FIFO) — measured as ~15us engine
            # holes at each hp boundary.
            norm_q = []
            tail_norms = []
            # per-v4 norm completion count for the final hp (gates tail
            # out-proj units)
            last_norms = [0, 0, 0, 0]
            # b0's blocks (mt 0..15) go out as interleaved halves under
            # b1's attention; the tail handles mt 16..31 as full units
            next_half = [0]
            next_unit = [16]

            def make_norm(scrr_row, oc_sb, ot, tok0, par, lastv4=None):
                def f():
                    ibc = ibc_pool.tile(
                        [64, 512], F32, tag="ibc", name="ibc"
                    )
                    nc.sync.dma_start(
                        out=ibc,
                        in_=scrr_row.partition_broadcast(64).squeeze(1),
                    )
                    if par == 0:
                        nc.vector.tensor_mul(
                            ot[0:64, tok0 : tok0 + 512], oc_sb[0:64, :], ibc
                        )
                    else:
                        on = on_pool.tile(
                            [64, 512], F16, tag="on", name="on"
                        )
                        nc.vector.tensor_mul(on, oc_sb[0:64, :], ibc)
                        nc.sync.dma_start(
                            out=ot[64:128, tok0 : tok0 + 512], in_=on
                        )
                    if lastv4 is not None:
                        last_norms[lastv4] += 1
                return f

            def emit_unit(act_free):
                if next_unit[0] < NTOK // 128:
                    out_proj(next_unit[0], act_free)
                    next_unit[0] += 1

            for b2 in range(B):
                if b2 == 1:
                    pump(999)  # drain any leftover b1-QKV units
                for hp in range(2):  # head pair (2hp, 2hp+1)
                    qtile = [qT01, qT23][hp]
                    scra = nc.dram_tensor(f"scra_{b2}_{hp}", [8, 512], F32)
                    scrr = nc.dram_tensor(f"scrr_{b2}_{hp}", [8, 512], F32)
                    oc_save = {}
                    for v4 in range(4):  # 512-query blocks
                        tq0 = 512 * v4
                        jmax = 4 * v4 + 4
                        # both heads in one single-bank-per-head accumulator
                        # tile; bufs=1 works because the tile is freed by an
                        # SBUF copy right after its last PV, and the deeper
                        # (3-buf) sps ring is what pipelines the chunks
                        ops2 = psB.tile([65, 2, 512], F32, tag="o")
                        for j in range(jmax):
                            tqs = max(128 * j, tq0)
                            W = tq0 + 512 - tqs
                            oc = tqs - tq0
                            ksl = slice(T * b2 + 128 * j, T * b2 + 128 * (j + 1))
                            qsl = slice(T * b2 + tqs, T * b2 + tqs + W)
                            diag = 128 * j >= tq0
                            sps = psA.tile([128, 2, 512], F32, tag="a")
                            # even head rows 0:64, odd head rows 64:128 —
                            # adjacent issue -> concurrent PE row groups
                            nc.tensor.matmul(
                                sps[:, 0, :W],
                                kvT[0:64, ksl], qtile[0:64, qsl],
                                start=True, stop=True,
                            )
                            nc.tensor.matmul(
                                sps[:, 1, :W],
                                ktdup[64:128, ksl], qtile[64:128, qsl],
                                start=True, stop=True,
                            )
                            es = es_pool.tile(
                                [128, 2, 512], F16, tag="ese", bufs=3
                            )
                            nc.scalar.activation(
                                out=es[:, :, :W], in_=sps[:, :, :W],
                                func=AF.Exp, scale=SCALE,
                            )
                            # causal mask for the diagonal block: zero the
                            # above-diagonal exps on the otherwise-idle
                            # gpsimd engine (the kernel is PE-bound; the
                            # extra exp->mask->PV stage pipelines through
                            # the 4-deep es ring)
                            if diag:
                                for par in range(2):
                                    nc.gpsimd.tensor_mul(
                                        es[:, par, 0:128],
                                        es[:, par, 0:128],
                                        dmask_sb,
                                    )
                            for par in range(2):
                                nc.tensor.matmul(
                                    ops2[:, par, oc : oc + W],
                                    vaug[:, b2 * NJ + j, :],
                                    es[:, par, :W],
                                    start=(j == 0),
                                    stop=(j == jmax - 1),
                                )
                            # Per j-step side-channel emissions: one dripped
                            # norm from the previous hp (only after a couple
                            # j-steps so its DRAM roundtrip is resolved and
                            # it can't head-of-line-block the DVE FIFO), and
                            # one b1-QKV filler unit during b0's attention.
                            if norm_q and (v4 > 0 or j >= 2):
                                norm_q.pop(0)()
                            if b2 == 0:
                                pump(1)
                            elif (
                                j % 2 == 1
                                and (hp == 1 or v4 >= 1)
                                and next_half[0] < 32
                            ):
                                nh = next_half[0]
                                out_proj_half(nh // 2, nh % 2, False)
                                next_half[0] += 1
                        # free the accumulator immediately: PV rows (and the
                        # ones-row rowsum) go to SBUF; the raw rowsums are
                        # stashed in DRAM for the batched hp-end reciprocal
                        for par in range(2):
                            oc_sb = ocp_pool.tile([65, 512], F32, tag="oc")
                            nc.vector.tensor_copy(oc_sb, ops2[0:65, par, :])
                            r = 2 * v4 + par
                            nc.sync.dma_start(
                                out=scra[r : r + 1, :], in_=oc_sb[64:65, :]
                            )
                            oc_save[(v4, par)] = oc_sb
                        # final hp: per-v4 rowsum reciprocal chain launched
                        # as soon as the v4's rowsums hit DRAM (it runs in
                        # the background on sync/ACT), but the normalize
                        # muls are held for the tail, where the roundtrip
                        # is guaranteed resolved.
                        if b2 == 1 and hp == 1:
                            rs2 = rows.tile([2, 512], F32, tag="lnr")
                            nc.sync.dma_start(
                                out=rs2, in_=scra[2 * v4 : 2 * v4 + 2, :]
                            )
                            nc.scalar.activation(
                                out=rs2, in_=rs2, func=AF.Ln
                            )
                            nc.scalar.activation(
                                out=rs2, in_=rs2, func=AF.Exp, scale=-1.0
                            )
                            nc.sync.dma_start(
                                out=scrr[2 * v4 : 2 * v4 + 2, :], in_=rs2
                            )
                            for par in range(2):
                                tail_norms.append(make_norm(
                                    scrr[2 * v4 + par : 2 * v4 + par + 1, :],
                                    oc_save[(v4, par)], OT[1],
                                    T + 512 * v4, par, lastv4=v4,
                                ))
                    if not (b2 == 1 and hp == 1):
                        # batched rowsum reciprocal chain for this hp; the
                        # 8 normalize closures drip into the NEXT hp's
                        # j-steps (their ibc roundtrips resolve meanwhile)
                        rsum8 = rows.tile([8, 512], F32, tag="lnr")
                        nc.sync.dma_start(out=rsum8, in_=scra[:, :])
                        nc.scalar.activation(
                            out=rsum8, in_=rsum8, func=AF.Ln
                        )
                        nc.scalar.activation(
                            out=rsum8, in_=rsum8, func=AF.Exp, scale=-1.0
                        )
                        nc.sync.dma_start(out=scrr[:, :], in_=rsum8)
                        for nv4 in range(4):
                            for par in range(2):
                                r = 2 * nv4 + par
                                norm_q.append(make_norm(
                                    scrr[r : r + 1, :],
                                    oc_save[(nv4, par)], OT[hp],
                                    T * b2 + 512 * nv4, par,
                                ))

            while norm_q:
                norm_q.pop(0)()

            # ---- tail: remaining out-projections, final-hp norms
            # threaded in: each b1 v4-block's norm pair lands just before
            # the units that read it.
            while next_half[0] < 32:
                nh = next_half[0]
                out_proj_half(nh // 2, nh % 2, True)
                next_half[0] += 1
            for v4t in range(4):
                tail_norms.pop(0)()
                tail_norms.pop(0)()
                for _ in range(4):
                    emit_unit(act_free=True)
    _split_waits(nc)
    return nc


def _host_inputs(x, sinusoidal_pos, Wq, bq, Wk, bk, Wv, bv, Wo):
    xT = np.ascontiguousarray(x.reshape(NTOK, C).T).astype(np.float16)

    sp = np.asarray(sinusoidal_pos, dtype=np.float32).reshape(T, D)
    cosd = np.repeat(sp[:, 0::2], 2, axis=1)     # [T, D]
    sind = np.repeat(sp[:, 1::2], 2, axis=1)
    cosb = np.tile(cosd.T, (1, B))               # [D, NTOK]
    sinb = np.tile(sind.T, (1, B))
    cos2 = np.ascontiguousarray(np.concatenate([cosb, cosb], 0))  # [128, NTOK]
    sin2 = np.ascontiguousarray(np.concatenate([sinb, sinb], 0))

    P = np.zeros((D, D), dtype=np.float32)
    P[: D // 2, D // 2 :] = np.eye(D // 2)
    P[D // 2 :, : D // 2] = -np.eye(D // 2)
    pmat = np.zeros((128, 128), dtype=np.float32)
    pmat[:64, :64] = P
    pmat[64:, 64:] = P

    f = np.arange(128)[None, :]
    p = np.arange(128)[:, None]
    # -1e4 where tk > tq in the S^T diag block; exp underflows to 0
    dneg = np.where(p > f, np.float32(-1e4), np.float32(0.0))

    ident2 = np.concatenate([np.eye(64), np.eye(64)], 0).astype(np.float32)

    shared = {
        "xT": xT, "cos2": cos2, "sin2": sin2,
        "pmat": pmat, "dneg": dneg,
        "id128": np.eye(128, dtype=np.float32),
        "dmask": (f >= p).astype(np.float16),
        "ident2": ident2,
        "vones": np.ones((128, 2 * (T // 128)), dtype=np.float16),
    }
    per_core = []
    for c in range(8):
        # q head h uses kv head h % KVH (jnp.tile), so core c owns
        # q heads {c, c+8, c+16, c+24} and kv head c.
        heads = [c + KVH * g for g in range(HPC)]
        qrows = np.concatenate([np.arange(D * h, D * (h + 1)) for h in heads])
        Wq_c = Wq[qrows]
        Wk_c = Wk[D * c : D * (c + 1)]
        Wv_c = Wv[D * c : D * (c + 1)]
        wcatT = np.ascontiguousarray(
            np.concatenate([Wq_c, Wk_c, Wv_c], 0).T
        ).astype(np.float16)
        bcat = np.concatenate(
            [bq[qrows], bk[D * c : D * (c + 1)], bv[D * c : D * (c + 1)]]
        ).astype(np.float32)
        bqkv = np.ascontiguousarray(bcat.reshape(3, 128).T)
        woR = np.ascontiguousarray(Wo[:, qrows].T).astype(np.float16)
        per_core.append(dict(shared, wcatT=wcatT, bqkv=bqkv, woR=woR))
    return per_core


def kernel(x, mask, sinusoidal_pos, Wq, bq, Wk, bk, Wv, bv, Wo, bo):
    x = np.asarray(x, dtype=np.float32)
    in_maps = _host_inputs(
        x, sinusoidal_pos,
        np.asarray(Wq, np.float32), np.asarray(bq, np.float32),
        np.asarray(Wk, np.float32), np.asarray(bk, np.float32),
        np.asarray(Wv, np.float32), np.asarray(bv, np.float32),
        np.asarray(Wo, np.float32),
    )
    if "nc" not in _NC_CACHE:
        _NC_CACHE["nc"] = build_nc()
    res = run_bass_kernel_spmd(
        _NC_CACHE["nc"], in_maps, core_ids=list(range(8))
    )
    y = np.zeros((NTOK, C), dtype=np.float32)
    for r in res.results:
        y += r["y"].astype(np.float32)
    y += np.asarray(bo, np.float32)[None, :]
    return y.reshape(B, T, C)



# revision 59
# speedup vs baseline: 1.0422x; 1.0422x over previous
"""Trainium2 Bass kernel for GQA multi-head attention with RoPE.

Sharding: tensor-parallel over heads. Core c owns q-heads 4c..4c+3 and
kv-head c. Each core computes its QKV projection slice, RoPE, causal
attention for its heads, and a partial output projection
(attn_out_local @ Wo[:, local].T). The host sums the 8 partial y's.

Device layouts (per core):
  xT      [C, B*T]   x transposed (replicated)
  wcatT   [C, 384]   [Wq_loc | Wk_loc | Wv_loc].T
  qT/kT   [d, tok]   head-dim on partitions ("T-layout")
  scores  S^T [tk partitions, tq free] so no transposes are needed:
          exp(S^T) feeds P@V directly as the moving operand with
          v_aug = [v | ones] stationary; the ones row yields softmax
          row-sums in partition 64 of the PV accumulator.
  y       [B*T, C]   partial; host adds the 8 partials + bo.

Matmul operands are float32r (fp32 with 11-bit mantissa, full PE rate
at N>=256); the QKV projection inputs (x, Wqkv) are fp16 (10-bit
mantissa, halves the HBM-bound x traffic). PSUM accumulation is fp32.
"""

import sys

sys.path.insert(0, "/opt/trn_rl_repo")

import numpy as np

import bass_rust
import concourse.bass as bass
import concourse.tile as tile
from concourse import mybir
from concourse.bass_utils import run_bass_kernel_spmd

B, T, C = 2, 2048, 2048
H, KVH, D = 32, 8, 64
NTOK = B * T                 # 4096
HPC = H // 8                 # 4 q heads per core
QL = HPC * D                 # 256 local q dims
KC = C // 128                # 16 contraction chunks
SCALE = float(D) ** -0.5

F32 = mybir.dt.float32
F32R = mybir.dt.float32r
F16 = mybir.dt.float16
AF = mybir.ActivationFunctionType

_NC_CACHE = {}


def _split_waits(nc, limit=1):
    """Walrus in this toolchain allows only one sync-wait per instruction.

    Tile emits instructions with several sem waits (drain/barrier, phase
    boundaries). Hoist the excess onto same-engine NoOps inserted right
    before the instruction — program order on the engine queue preserves
    the wait semantics.
    """
    ctr = 0
    for f in nc.m.functions:
        for blk in f.blocks:
            out = []
            changed = False
            for inst in list(blk.instructions):
                si = inst.sync_info
                if si is not None and len(si.on_wait) > limit:
                    waits = list(si.on_wait)
                    keep, excess = waits[:limit], waits[limit:]
                    for i in range(0, len(excess), limit):
                        ctr += 1
                        nop = mybir.InstNoOp(
                            name=f"I-wsplit-{ctr}", ins=[], outs=[]
                        )
                        nop.engine = inst.engine
                        nop.sync_info = bass_rust.SyncInfo(
                            on_wait=excess[i : i + limit], on_update=[]
                        )
                        out.append(nop)
                        changed = True
                    inst.sync_info = bass_rust.SyncInfo(
                        on_wait=keep, on_update=list(si.on_update)
                    )
                out.append(inst)
            if changed:
                blk.instructions = out
    return ctr


def build_nc():
    nc = bass.Bass(trn_type="TRN2")

    xT_d = nc.dram_tensor("xT", [C, NTOK], F16, kind="ExternalInput")
    wcat_d = nc.dram_tensor("wcatT", [C, 384], F16, kind="ExternalInput")
    bias_d = nc.dram_tensor("bqkv", [128, 3], F32, kind="ExternalInput")
    wo_d = nc.dram_tensor("woR", [QL, C], F16, kind="ExternalInput")
    cos_d = nc.dram_tensor("cos2", [128, NTOK], F32R, kind="ExternalInput")
    sin_d = nc.dram_tensor("sin2", [128, NTOK], F32R, kind="ExternalInput")
    pmat_d = nc.dram_tensor("pmat", [128, 128], F32R, kind="ExternalInput")
    dneg_d = nc.dram_tensor("dneg", [128, 128], F32R, kind="ExternalInput")
    id128_d = nc.dram_tensor("id128", [128, 128], F32R, kind="ExternalInput")
    ident_d = nc.dram_tensor("ident2", [128, 64], F32R, kind="ExternalInput")
    vones_d = nc.dram_tensor(
        "vones", [128, 2 * (T // 128)], F16, kind="ExternalInput"
    )
    y_d = nc.dram_tensor("y", [NTOK, C], F16, kind="ExternalOutput")

    with tile.TileContext(nc) as tc:
        with (
            tc.tile_pool(name="consts", bufs=1) as consts,
            tc.tile_pool(name="xs", bufs=8) as xs_pool,
            tc.tile_pool(name="acts", bufs=1) as acts,
            tc.tile_pool(name="big", bufs=2) as big,
            tc.tile_pool(name="tmp", bufs=2) as tmp_pool,
            tc.tile_pool(name="es", bufs=3) as es_pool,
            tc.tile_pool(name="rows", bufs=2) as rows,
            tc.tile_pool(name="ibc", bufs=3) as ibc_pool,
            tc.tile_pool(name="ocp", bufs=10) as ocp_pool,
            tc.tile_pool(name="onorm", bufs=1) as on_pool,
            tc.tile_pool(name="ystage", bufs=2) as y_pool,
            tc.tile_pool(name="psA", bufs=2, space="PSUM") as psA,
            tc.tile_pool(name="psB", bufs=1, space="PSUM") as psB,
            tc.tile_pool(name="fil", bufs=1, space="PSUM") as fil,
        ):
            # ---- constants needed by phase 1's first group ----
            # wo / mbias / cos / sin are deliberately NOT loaded here: the
            # first QKV matmul stalls on everything queued ahead of its xt
            # tile, so only group-0 prerequisites go first. wcat is one tile
            # per k chunk so matmul k waits on exactly its own DMA.
            wcat_ks = [
                consts.tile([128, 384], F16, tag=f"wc{k}", name=f"wcat{k}")
                for k in range(KC)
            ]
            bias_sb = consts.tile([128, 3], F32, tag="bias")
            pmat_sb = consts.tile([128, 128], F32R, tag="pmat")
            ident_sb = consts.tile([128, 64], F32R, tag="ident")

            wo_sb = consts.tile([128, 2, C], F16, tag="wo")
            dneg_sb = consts.tile([128, 128], F32R, tag="dneg")
            id128_sb = consts.tile([128, 128], F32R, tag="id128")

            qT01 = acts.tile([128, NTOK], F32R, tag="qT01")
            qT23 = acts.tile([128, NTOK], F32R, tag="qT23")
            kvT = acts.tile([128, NTOK], F32R, tag="kvT")
            ktdup = acts.tile([128, NTOK], F32R, tag="ktdup")
            vaug = acts.tile([128, 2 * (T // 128), 65], F16, tag="vaug")

            def cs_pair(tok0, width):
                cg = big.tile([128, 1024], F32R, tag="cs", bufs=4,
                              name="cosg")
                sg = big.tile([128, 1024], F32R, tag="cs", bufs=4,
                              name="sing")
                nc.sync.dma_start(
                    out=cg[:, :width], in_=cos_d[:, tok0 : tok0 + width]
                )
                nc.sync.dma_start(
                    out=sg[:, :width], in_=sin_d[:, tok0 : tok0 + width]
                )
                return cg[:, :width], sg[:, :width]

            # ---- phase 0: PE warmup ----
            # ~40 back-to-back tiny fp16 matmuls on a memset tile keep the
            # PE HAM activity window busy from t=0, so the first real QKV
            # matmuls run at 2.4 GHz instead of the cold 1.2 GHz ramp.
            wtile = consts.tile([128, 128], F16, tag="warm")
            nc.vector.memset(wtile, 0.0)
            wps = psA.tile([128, 128], F32, tag="a")
            for _ in range(40):
                nc.tensor.matmul(
                    wps, wtile, wtile, start=True, stop=True
                )

            # ---- phase 1: QKV projection + RoPE + v-transpose, fused ----
            # Processing 1024-token groups keeps the PE stream dense: the
            # RoPE rotate-matmuls and v-transposes of group g interleave
            # with the QKV matmuls of group g+1, so the HAM never
            # re-throttles between phases.
            qkv_dst = [qT01, qT23, kvT]
            NJ = T // 128  # 16
            for ng in range(2):  # b0 groups only; b1 QKV is interleaved
                base = 1024 * ng  # under attention-b0 (see gen_b1_qkv)
                ps0 = psA.tile([128, 1024], F32, tag="a")
                ps1 = psA.tile([128, 1024], F32, tag="a")
                ps2 = psB.tile([128, 1024], F32, tag="o")
                pss = [ps0, ps1, ps2]
                for k2 in range(KC // 2):
                    if ng == 0:
                        for k in (2 * k2, 2 * k2 + 1):
                            nc.sync.dma_start(
                                out=wcat_ks[k],
                                in_=wcat_d[128 * k : 128 * (k + 1), :],
                            )
                    xt2 = xs_pool.tile([128, 2, 1024], F16, tag="xs", bufs=4)
                    for kk in range(2):
                        k = 2 * k2 + kk
                        nc.sync.dma_start(
                            out=xt2[:, kk, :],
                            in_=xT_d[128 * k : 128 * (k + 1),
                                     base : base + 1024],
                        )
                    for kk in range(2):
                        k = 2 * k2 + kk
                        for m in range(3):
                            for c2 in range(2):
                                nc.tensor.matmul(
                                    pss[m][:, 512 * c2 : 512 * (c2 + 1)],
                                    wcat_ks[k][:, 128 * m : 128 * (m + 1)],
                                    xt2[:, kk, 512 * c2 : 512 * (c2 + 1)],
                                    start=(k == 0),
                                    stop=(k == KC - 1),
                                )
                cos_g, sin_g = cs_pair(base, 1024)
                if ng == 0:
                    nc.sync.dma_start(out=bias_sb, in_=bias_d[:, :])
                    nc.sync.dma_start(out=pmat_sb, in_=pmat_d[:, :])
                    nc.sync.dma_start(out=ident_sb, in_=ident_d[:, :])
                    nc.sync.dma_start(
                        out=vaug[:, :, 64:65], in_=vones_d[:, :].unsqueeze(2)
                    )
                for m in range(3):
                    nc.scalar.activation(
                        out=qkv_dst[m][:, base : base + 1024],
                        in_=pss[m],
                        func=AF.Identity,
                        bias=bias_sb[:, m : m + 1],
                        scale=1.0,
                    )
                # RoPE for this token group (token-pointwise)
                for dst, rn in ((qT01, 128), (qT23, 128), (kvT, 64)):
                    rot = psA.tile([128, 1024], F32, tag="a")
                    for c2 in range(2):
                        nc.tensor.matmul(
                            rot[:rn, 512 * c2 : 512 * (c2 + 1)],
                            pmat_sb[:rn, :rn],
                            dst[:rn, base + 512 * c2 : base + 512 * (c2 + 1)],
                            start=True,
                            stop=True,
                        )
                    tmp = tmp_pool.tile([128, 1024], F32, tag="tmp")
                    nc.vector.tensor_mul(
                        tmp[:rn], rot[:rn, :], sin_g[:rn, :]
                    )
                    nc.vector.tensor_mul(
                        dst[:rn, base : base + 1024],
                        dst[:rn, base : base + 1024],
                        cos_g[:rn, :],
                    )
                    nc.vector.tensor_add(
                        dst[:rn, base : base + 1024],
                        dst[:rn, base : base + 1024],
                        tmp[:rn],
                    )
                # duplicate this group's roped kT into partitions 64:128
                # (overlaps with the next group instead of one serial copy)
                nc.sync.dma_start(
                    out=ktdup[64:128, base : base + 1024],
                    in_=kvT[0:64, base : base + 1024],
                )
                # v transposes for this token group (v is not roped)
                b2 = ng // 2
                for jj in range(8):
                    jt = (ng % 2) * 8 + jj
                    vps = psB.tile([128, 64], F32R, tag="o")
                    nc.tensor.transpose(
                        vps,
                        kvT[64:128, T * b2 + 128 * jt : T * b2 + 128 * (jt + 1)],
                        ident_sb[64:128, :],
                    )
                    # ACT is idle during phase 1; keep DVE free for RoPE
                    nc.scalar.copy(vaug[:, b2 * NJ + jt, 0:64], vps)

            # attention/phase-4 constants: queued behind phase 1's loads so
            # they never delay the first QKV matmuls
            nc.sync.dma_start(out=dneg_sb, in_=dneg_d[:, :])
            nc.sync.dma_start(out=id128_sb, in_=id128_d[:, :])
            for k in range(2):
                nc.sync.dma_start(
                    out=wo_sb[:, k, :], in_=wo_d[128 * k : 128 * (k + 1), :]
                )

            # ---- phase 3: attention, per (batch, head, tq-half) ----
            OT0 = big.tile([128, NTOK], F16, tag="ot0", bufs=1)
            OT1 = big.tile([128, NTOK], F16, tag="ot1", bufs=1)
            OT = [OT0, OT1]

            def gen_b1_qkv():
                # b1's QKV+RoPE+v-transpose as a stream of small emission
                # units, pumped one per attention-b0 j-step so the PE FIFO
                # alternates [attention j][qkv chunk] and stays dense while
                # ACT grinds the exp pipeline. PSUM: dedicated 2-bank tag
                # ("f"), two-pass per 512-token halfgroup (q01/q23 pass,
                # then kv pass re-reading the same xt tiles).
                for hb in range(4):
                    tok0 = T + 512 * hb
                    cos_h, sin_h = cs_pair(tok0, 512)
                    fq = fil.tile([128, 2, 512], F32, tag="f", name=f"fq{hb}")
                    xts = []
                    for cu in range(8):
                        xt2 = xs_pool.tile(
                            [128, 2, 512], F16, tag="xsb", bufs=8,
                            name=f"xb{hb}_{cu}",
                        )
                        for kk in range(2):
                            k = 2 * cu + kk
                            nc.sync.dma_start(
                                out=xt2[:, kk, :],
                                in_=xT_d[128 * k : 128 * (k + 1),
                                         tok0 : tok0 + 512],
                            )
                        xts.append(xt2)
                        for kk in range(2):
                            k = 2 * cu + kk
                            for m in range(2):
                                nc.tensor.matmul(
                                    fq[:, m, :],
                                    wcat_ks[k][:, 128 * m : 128 * (m + 1)],
                                    xt2[:, kk, :],
                                    start=(k == 0), stop=(k == KC - 1),
                                )
                        yield
                    for m in range(2):
                        nc.vector.tensor_scalar_add(
                            [qT01, qT23][m][:, tok0 : tok0 + 512],
                            fq[:, m, :],
                            bias_sb[:, m : m + 1],
                        )
                    yield
                    fkv = fil.tile(
                        [128, 2, 512], F32, tag="f", name=f"fkv{hb}"
                    )
                    for cu in range(8):
                        for kk in range(2):
                            k = 2 * cu + kk
                            nc.tensor.matmul(
                                fkv[:, 0, :],
                                wcat_ks[k][:, 256:384],
                                xts[cu][:, kk, :],
                                start=(k == 0), stop=(k == KC - 1),
                            )
                        if cu % 2 == 1:
                            yield
                    nc.vector.tensor_scalar_add(
                        kvT[:, tok0 : tok0 + 512],
                        fkv[:, 0, :],
                        bias_sb[:, 2:3],
                    )
                    yield
                    # v transposes (v is not roped; kvT[64:128] = v)
                    for vi in range(2):
                        vt = fil.tile(
                            [128, 2, 64], F32R, tag="f", name=f"vt{hb}{vi}"
                        )
                        for w2 in range(2):
                            jt = 4 * hb + 2 * vi + w2
                            nc.tensor.transpose(
                                vt[:, w2, :],
                                kvT[64:128, T + 128 * jt : T + 128 * (jt + 1)],
                                ident_sb[64:128, :],
                            )
                            nc.vector.tensor_copy(
                                vaug[:, NJ + jt, 0:64], vt[:, w2, :]
                            )
                        yield
                    # RoPE for q01/q23
                    rotq = fil.tile(
                        [128, 2, 512], F32, tag="f", name=f"rq{hb}"
                    )
                    for m in range(2):
                        nc.tensor.matmul(
                            rotq[:, m, :],
                            pmat_sb,
                            [qT01, qT23][m][:, tok0 : tok0 + 512],
                            start=True, stop=True,
                        )
                    yield
                    for m in range(2):
                        dst = [qT01, qT23][m]
                        tmp = tmp_pool.tile([128, 512], F32, tag="tmp")
                        nc.vector.tensor_mul(tmp, rotq[:, m, :], sin_h)
                        nc.vector.tensor_mul(
                            dst[:, tok0 : tok0 + 512],
                            dst[:, tok0 : tok0 + 512], cos_h,
                        )
                        nc.vector.tensor_add(
                            dst[:, tok0 : tok0 + 512],
                            dst[:, tok0 : tok0 + 512], tmp,
                        )
                        yield
                    # RoPE for k (rows 0:64 of kvT)
                    rotk = fil.tile([128, 512], F32, tag="f", name=f"rk{hb}")
                    nc.tensor.matmul(
                        rotk[:64, :], pmat_sb[:64, :64],
                        kvT[0:64, tok0 : tok0 + 512],
                        start=True, stop=True,
                    )
                    yield
                    tmp = tmp_pool.tile([128, 512], F32, tag="tmp")
                    nc.vector.tensor_mul(
                        tmp[:64], rotk[:64, :], sin_h[:64, :]
                    )
                    nc.vector.tensor_mul(
                        kvT[0:64, tok0 : tok0 + 512],
                        kvT[0:64, tok0 : tok0 + 512], cos_h[:64, :],
                    )
                    nc.vector.tensor_add(
                        kvT[0:64, tok0 : tok0 + 512],
                        kvT[0:64, tok0 : tok0 + 512], tmp[:64],
                    )
                    yield
                    nc.sync.dma_start(
                        out=ktdup[64:128, tok0 : tok0 + 512],
                        in_=kvT[0:64, tok0 : tok0 + 512],
                    )
                    yield

            b1_units = gen_b1_qkv()

            def pump(n=1):
                for _ in range(n):
                    if next(b1_units, "done") == "done":
                        break

            half_ys = {}

            def out_proj_half(mt, half, act_free):
                # Half of one token block's output projection: two 512-col
                # chunks accumulated in a 2-bank "f" tile (free outside the
                # b1-QKV stretch), copies to a shared staging tile, and on
                # the second half one [128, 2048] DMA. Interleaved halves
                # are emitted at separate attention j-steps so the f-ring's
                # draw-waits-copy dependency hides behind an attention step
                # instead of stalling the PE FIFO.
                if half == 0:
                    ys = y_pool.tile([128, C], F16, tag="ys")
                    half_ys[mt] = ys
                else:
                    ys = half_ys.pop(mt)
                fp = fil.tile(
                    [128, 2, 512], F32, tag="f", name=f"yp{mt}_{half}"
                )
                for c2 in range(2):
                    ncc = 2 * half + c2
                    for kq in range(2):
                        nc.tensor.matmul(
                            fp[:, c2, :],
                            OT[kq][:, 128 * mt : 128 * (mt + 1)],
                            wo_sb[:, kq, 512 * ncc : 512 * (ncc + 1)],
                            start=(kq == 0),
                            stop=(kq == 1),
                        )
                for c2 in range(2):
                    ncc = 2 * half + c2
                    ysl = ys[:, 512 * ncc : 512 * (ncc + 1)]
                    if act_free and c2 == 0:
                        nc.scalar.copy(ysl, fp[:, c2, :])
                    else:
                        nc.vector.tensor_copy(ysl, fp[:, c2, :])
                if half == 1:
                    nc.sync.dma_start(
                        out=y_d[128 * mt : 128 * (mt + 1), :], in_=ys,
                    )

            def out_proj(mt, act_free):
                # full-unit path for the tail: psA's "a" ring has no sps
                # draws there, so the chunk accumulators double-buffer
                ys = y_pool.tile([128, C], F16, tag="ys")
                for ncc in range(C // 512):
                    yp = psA.tile([128, 512], F32, tag="a")
                    for kq in range(2):
                        nc.tensor.matmul(
                            yp,
                            OT[kq][:, 128 * mt : 128 * (mt + 1)],
                            wo_sb[:, kq, 512 * ncc : 512 * (ncc + 1)],
                            start=(kq == 0),
                            stop=(kq == 1),
                        )
                    ysl = ys[:, 512 * ncc : 512 * (ncc + 1)]
                    # GPSIMD cannot access PSUM, so the copies are split
                    # between ACT and DVE in the tail, DVE-only while ACT
                    # is saturated with attention exps.
                    if act_free and ncc % 2 == 0:
                        nc.scalar.copy(ysl, yp)
                    else:
                        nc.vector.tensor_copy(ysl, yp)
                nc.sync.dma_start(
                    out=y_d[128 * mt : 128 * (mt + 1), :], in_=ys,
                )

            # Normalize ops are queued as small closures and dripped out
            # one per attention j-step: an 8-norm burst emitted right at an
            # hp boundary waits multi-us on the rowsum DRAM roundtrip and
            # head-of-line-blocks the DVE FIFO (and, through the first
            # out-proj matmul, the PE FIFO) — measured as ~15us engine
            # holes at each hp boundary.
            norm_q = []
            tail_norms = []
            # per-v4 norm completion count for the final hp (gates tail
            # out-proj units)
            last_norms = [0, 0, 0, 0]
            # b0's blocks (mt 0..15) go out as interleaved halves under
            # b1's attention; the tail handles mt 16..31 as full units
            next_half = [0]
            next_unit = [16]

            def make_norm(scrr_row, oc_sb, ot, tok0, par, lastv4=None):
                def f():
                    ibc = ibc_pool.tile(
                        [64, 512], F32, tag="ibc", name="ibc"
                    )
                    nc.sync.dma_start(
                        out=ibc,
                        in_=scrr_row.partition_broadcast(64).squeeze(1),
                    )
                    if par == 0:
                        nc.vector.tensor_mul(
                            ot[0:64, tok0 : tok0 + 512], oc_sb[0:64, :], ibc
                        )
                    else:
                        on = on_pool.tile(
                            [64, 512], F16, tag="on", name="on"
                        )
                        nc.vector.tensor_mul(on, oc_sb[0:64, :], ibc)
                        nc.sync.dma_start(
                            out=ot[64:128, tok0 : tok0 + 512], in_=on
                        )
                    if lastv4 is not None:
                        last_norms[lastv4] += 1
                return f

            def emit_unit(act_free):
                if next_unit[0] < NTOK // 128:
                    out_proj(next_unit[0], act_free)
                    next_unit[0] += 1

            for b2 in range(B):
                if b2 == 1:
                    pump(999)  # drain any leftover b1-QKV units
                for hp in range(2):  # head pair (2hp, 2hp+1)
                    qtile = [qT01, qT23][hp]
                    scra = nc.dram_tensor(f"scra_{b2}_{hp}", [8, 512], F32)
                    scrr = nc.dram_tensor(f"scrr_{b2}_{hp}", [8, 512], F32)
                    oc_save = {}
                    for v4 in range(4):  # 512-query blocks
                        tq0 = 512 * v4
                        jmax = 4 * v4 + 4
                        # both heads in one single-bank-per-head accumulator
                        # tile; bufs=1 works because the tile is freed by an
                        # SBUF copy right after its last PV, and the deeper
                        # (3-buf) sps ring is what pipelines the chunks
                        ops2 = psB.tile([65, 2, 512], F32, tag="o")
                        for j in range(jmax):
                            tqs = max(128 * j, tq0)
                            W = tq0 + 512 - tqs
                            oc = tqs - tq0
                            ksl = slice(T * b2 + 128 * j, T * b2 + 128 * (j + 1))
                            qsl = slice(T * b2 + tqs, T * b2 + tqs + W)
                            diag = 128 * j >= tq0
                            sps = psA.tile([128, 2, 512], F32, tag="a")
                            # even head rows 0:64, odd head rows 64:128 —
                            # adjacent issue -> concurrent PE row groups
                            nc.tensor.matmul(
                                sps[:, 0, :W],
                                kvT[0:64, ksl], qtile[0:64, qsl],
                                start=True, stop=not diag,
                            )
                            nc.tensor.matmul(
                                sps[:, 1, :W],
                                ktdup[64:128, ksl], qtile[64:128, qsl],
                                start=True, stop=not diag,
                            )
                            # Causal masking folded into the scores PSUM
                            # accumulation: for the diagonal block (its
                            # first 128 columns, where tq-tqs < tk-128j is
                            # possible) accumulate -1e4 * [tk > tq] via
                            # I128.T @ dneg, so exp underflows to 0 and no
                            # elementwise mask op sits between exp and PV.
                            if diag:
                                for par in range(2):
                                    nc.tensor.matmul(
                                        sps[:, par, 0:128],
                                        id128_sb, dneg_sb,
                                        start=False, stop=True,
                                    )
                            es = es_pool.tile([128, 2, 512], F16, tag="ese")
                            nc.scalar.activation(
                                out=es[:, :, :W], in_=sps[:, :, :W],
                                func=AF.Exp, scale=SCALE,
                            )
                            for par in range(2):
                                nc.tensor.matmul(
                                    ops2[:, par, oc : oc + W],
                                    vaug[:, b2 * NJ + j, :],
                                    es[:, par, :W],
                                    start=(j == 0),
                                    stop=(j == jmax - 1),
                                )
                            # Per j-step side-channel emissions: one dripped
                            # norm from the previous hp (only after a couple
                            # j-steps so its DRAM roundtrip is resolved and
                            # it can't head-of-line-block the DVE FIFO), and
                            # one b1-QKV filler unit during b0's attention.
                            if norm_q and (v4 > 0 or j >= 2):
                                norm_q.pop(0)()
                            if b2 == 0:
                                pump(1)
                            elif (
                                j % 2 == 1
                                and (hp == 1 or v4 >= 1)
                                and next_half[0] < 32
                            ):
                                nh = next_half[0]
                                out_proj_half(nh // 2, nh % 2, False)
                                next_half[0] += 1
                        # free the accumulator immediately: PV rows (and the
                        # ones-row rowsum) go to SBUF; the raw rowsums are
                        # stashed in DRAM for the batched hp-end reciprocal
                        for par in range(2):
                            oc_sb = ocp_pool.tile([65, 512], F32, tag="oc")
                            nc.vector.tensor_copy(oc_sb, ops2[0:65, par, :])
                            r = 2 * v4 + par
                            nc.sync.dma_start(
                                out=scra[r : r + 1, :], in_=oc_sb[64:65, :]
                            )
                            oc_save[(v4, par)] = oc_sb
                        # final hp: per-v4 rowsum reciprocal chain launched
                        # as soon as the v4's rowsums hit DRAM (it runs in
                        # the background on sync/ACT), but the normalize
                        # muls are held for the tail, where the roundtrip
                        # is guaranteed resolved.
                        if b2 == 1 and hp == 1:
                            rs2 = rows.tile([2, 512], F32, tag="lnr")
                            nc.sync.dma_start(
                                out=rs2, in_=scra[2 * v4 : 2 * v4 + 2, :]
                            )
                            nc.scalar.activation(
                                out=rs2, in_=rs2, func=AF.Ln
                            )
                            nc.scalar.activation(
                                out=rs2, in_=rs2, func=AF.Exp, scale=-1.0
                            )
                            nc.sync.dma_start(
                                out=scrr[2 * v4 : 2 * v4 + 2, :], in_=rs2
                            )
                            for par in range(2):
                                tail_norms.append(make_norm(
                                    scrr[2 * v4 + par : 2 * v4 + par + 1, :],
                                    oc_save[(v4, par)], OT[1],
                                    T + 512 * v4, par, lastv4=v4,
                                ))
                    if not (b2 == 1 and hp == 1):
                        # batched rowsum reciprocal chain for this hp; the
                        # 8 normalize closures drip into the NEXT hp's
                        # j-steps (their ibc roundtrips resolve meanwhile)
                        rsum8 = rows.tile([8, 512], F32, tag="lnr")
                        nc.sync.dma_start(out=rsum8, in_=scra[:, :])
                        nc.scalar.activation(
                            out=rsum8, in_=rsum8, func=AF.Ln
                        )
                        nc.scalar.activation(
                            out=rsum8, in_=rsum8, func=AF.Exp, scale=-1.0
                        )
                        nc.sync.dma_start(out=scrr[:, :], in_=rsum8)
                        for nv4 in range(4):
                            for par in range(2):
                                r = 2 * nv4 + par
                                norm_q.append(make_norm(
                                    scrr[r : r + 1, :],
                                    oc_save[(nv4, par)], OT[hp],
                                    T * b2 + 512 * nv4, par,
                                ))

            while norm_q:
                norm_q.pop(0)()

            # ---- tail: remaining out-projections, final-hp norms
            # threaded in: each b1 v4-block's norm pair lands just before
            # the units that read it.
            while next_half[0] < 32:
                nh = next_half[0]
                out_proj_half(nh // 2, nh % 2, True)
                next_half[0] += 1
            for v4t in range(4):
                tail_norms.pop(0)()
                tail_norms.pop(0)()
                for _ in range(4):
                    emit_unit(act_free=True)
    _split_waits(nc)
    return nc


def _host_inputs(x, sinusoidal_pos, Wq, bq, Wk, bk, Wv, bv, Wo):
    xT = np.ascontiguousarray(x.reshape(NTOK, C).T).astype(np.float16)

    sp = np.asarray(sinusoidal_pos, dtype=np.float32).reshape(T, D)
    cosd = np.repeat(sp[:, 0::2], 2, axis=1)     # [T, D]
    sind = np.repeat(sp[:, 1::2], 2, axis=1)
    cosb = np.tile(cosd.T, (1, B))               # [D, NTOK]
    sinb = np.tile(sind.T, (1, B))
    cos2 = np.ascontiguousarray(np.concatenate([cosb, cosb], 0))  # [128, NTOK]
    sin2 = np.ascontiguousarray(np.concatenate([sinb, sinb], 0))

    P = np.zeros((D, D), dtype=np.float32)
    P[: D // 2, D // 2 :] = np.eye(D // 2)
    P[D // 2 :, : D // 2] = -np.eye(D // 2)
    pmat = np.zeros((128, 128), dtype=np.float32)
    pmat[:64, :64] = P
    pmat[64:, 64:] = P

    f = np.arange(128)[None, :]
    p = np.arange(128)[:, None]
    # -1e4 where tk > tq in the S^T diag block; exp underflows to 0
    dneg = np.where(p > f, np.float32(-1e4), np.float32(0.0))

    ident2 = np.concatenate([np.eye(64), np.eye(64)], 0).astype(np.float32)

    shared = {
        "xT": xT, "cos2": cos2, "sin2": sin2,
        "pmat": pmat, "dneg": dneg,
        "id128": np.eye(128, dtype=np.float32),
        "ident2": ident2,
        "vones": np.ones((128, 2 * (T // 128)), dtype=np.float16),
    }
    per_core = []
    for c in range(8):
        # q head h uses kv head h % KVH (jnp.tile), so core c owns
        # q heads {c, c+8, c+16, c+24} and kv head c.
        heads = [c + KVH * g for g in range(HPC)]
        qrows = np.concatenate([np.arange(D * h, D * (h + 1)) for h in heads])
        Wq_c = Wq[qrows]
        Wk_c = Wk[D * c : D * (c + 1)]
        Wv_c = Wv[D * c : D * (c + 1)]
        wcatT = np.ascontiguousarray(
            np.concatenate([Wq_c, Wk_c, Wv_c], 0).T
        ).astype(np.float16)
        bcat = np.concatenate(
            [bq[qrows], bk[D * c : D * (c + 1)], bv[D * c : D * (c + 1)]]
        ).astype(np.float32)
        bqkv = np.ascontiguousarray(bcat.reshape(3, 128).T)
        woR = np.ascontiguousarray(Wo[:, qrows].T).astype(np.float16)
        per_core.append(dict(shared, wcatT=wcatT, bqkv=bqkv, woR=woR))
    return per_core


def kernel(x, mask, sinusoidal_pos, Wq, bq, Wk, bk, Wv, bv, Wo, bo):
    x = np.asarray(x, dtype=np.float32)
    in_maps = _host_inputs(
        x, sinusoidal_pos,
        np.asarray(Wq, np.float32), np.asarray(bq, np.float32),
        np.asarray(Wk, np.float32), np.asarray(bk, np.float32),
        np.asarray(Wv, np.float32), np.asarray(bv, np.float32),
        np.asarray(Wo, np.float32),
    )
    if "nc" not in _NC_CACHE:
        _NC_CACHE["nc"] = build_nc()
    res = run_bass_kernel_spmd(
        _NC_CACHE["nc"], in_maps, core_ids=list(range(8))
    )
    y = np.zeros((NTOK, C), dtype=np.float32)
    for r in res.results:
        y += r["y"].astype(np.float32)
    y += np.asarray(bo, np.float32)[None, :]
    return y.reshape(B, T, C)

